# revision 3
# baseline (speedup 1.0000x reference)
"""Trainium2 Bass kernel for nn_DOAM (dense CNN attention module).

Single fused device phase, pure data parallel (4 images/core x 8 cores):
  conv1..conv5 -> x (device DRAM, fp16)
  on-device 5/10/15 block-average pools (H-pool via matmul, W-pool via
  strided-view reduce), nearest upsample (X via broadcast copy, Y via
  strided DRAM DMAs)
  c5/c10/c15 concat convs, gated conv, BN partial sums
  cross-core AllReduce of the [8,2] channel stats, BN fold on device
  8->1 conv + sigmoid + residual blend with the fp16 input image

Convs are "row-batched banded GEMMs": for a group of R output rows,
M = R*Cout output partitions, K = (R+2)*Cin input partitions and the 3
kernel-x taps are 3 matmuls accumulating in PSUM with shifted rhs column
windows.  All matmul operands fp16 (stats path fp32), PSUM fp32.

Dispatch: a cached jax.jit(shard_map) over the 8 axon devices; donated
output buffers are created on-device so only im (fp16) goes up and the
fp16 blended output comes back.
"""
import sys
import numpy as np
from contextlib import ExitStack

sys.path.insert(0, "/opt/trn_rl_repo")
import concourse.bacc as bacc
import concourse.tile as tile
from concourse import mybir
from concourse import bass2jax

F16 = mybir.dt.float16
F32 = mybir.dt.float32
U8 = mybir.dt.uint8
AF = mybir.ActivationFunctionType
ALU = mybir.AluOpType

H = W = 300
HP = WP = 302
NCORES = 8
BPC = 4          # images per core
EPS = 1e-5
NPIX = 32.0 * H * W
S_IM = 11.0 / 255.0   # u8 image quantization step (zero at code 128)

_CACHE = {}

# all fp16 weight tensors, in packing order: name -> [K, C]
F16_SPECS = [
    ("w_l1", 24, 144), ("w_l2", 64, 288),
    ("w_l3a", 128, 288), ("w_l3b", 128, 288),
    ("w_l4a", 128, 288), ("w_l4b", 128, 288), ("w_l5", 128, 144),
    ("w_c5x", 112, 288), ("w_c5u", 112, 288),
    ("w_c10x", 112, 288), ("w_c10u", 112, 288),
    ("w_c15x", 112, 288), ("w_c15u", 112, 288),
    ("w_wg0", 112, 288), ("w_wg1", 112, 288), ("w_wg2", 112, 288),
    ("w_wm0", 112, 288), ("w_wm1", 112, 288), ("w_wm2", 112, 288),
    ("pm_p5", 120, 24), ("pm_p10a", 120, 24), ("pm_p10b", 120, 24),
    ("pm_p15a", 120, 16), ("pm_p15b", 120, 16),
    ("w_l11", 112, 36),
]
_TOT16 = sum(k * c for _, k, c in F16_SPECS)
PACK_S = (_TOT16 + NCORES - 1) // NCORES
PACK_S += (-PACK_S) % 8
F16_OFFS = {}
_o = 0
for _nm, _k, _c in F16_SPECS:
    F16_OFFS[_nm] = _o
    _o += _k * _c

# all fp32 side inputs, packed into one replicated tensor
F32_SPECS = [
    ("b_l1", 48, 1), ("b_l2", 96, 1), ("b_l3a", 96, 1), ("b_l3b", 96, 1),
    ("b_l4", 96, 1), ("b_l5", 48, 1),
    ("b_c5", 96, 1), ("b_c10", 96, 1), ("b_c15", 96, 1),
    ("b_wg", 96, 1), ("b_wm", 96, 1),
    ("red96", 96, 8), ("bc112", 8, 336), ("bnwb", 8, 2), ("b11t", 12, 1),
]
PACK32_S = sum(k * c for _, k, c in F32_SPECS)
F32_OFFS = {}
_o = 0
for _nm, _k, _c in F32_SPECS:
    F32_OFFS[_nm] = _o
    _o += _k * _c


def band_lhs(w, R, cin_idx, cout_idx, perm=False):
    """w [O,I,3,3] -> [K=(R+2)*len(cin), 3*M] fp32, M=R*len(cout).

    Window position p (0 = top halo, 1..R interior, R+1 = bottom halo) maps to
    partition row-block p (natural, DMA-fed panels) or, when perm=True,
    interior first: p in 1..R -> p-1, p==0 -> R, p==R+1 -> R+1."""
    w = np.asarray(w, np.float32)[np.ix_(list(cout_idx), list(cin_idx))]
    Ob, Cb = w.shape[:2]
    K, M = (R + 2) * Cb, R * Ob
    lhs = np.zeros((K, 3, M), np.float32)
    yo = np.arange(R)
    for dy in range(3):
        p = yo + dy
        blk = np.where(p == 0, R, np.where(p <= R, p - 1, p)) if perm else p
        cols = (yo * Ob)[:, None] + np.arange(Ob)[None, :]
        for ci in range(Cb):
            lhs[(blk * Cb + ci)[:, None], :, cols] = \
                np.broadcast_to(w[:, ci, dy, :], (R, Ob, 3))
    return np.ascontiguousarray(lhs.reshape(K, 3 * M))


def tile_bias(b, R):
    return np.tile(np.asarray(b, np.float32), R)[:, None]  # [R*O, 1]


def pool_mats():
    """H-pool matmul lhsT matrices, fp16, 1/k^2 folded in."""
    p5 = np.zeros((120, 24), np.float32)
    p10a = np.zeros((120, 24), np.float32)
    p10b = np.zeros((120, 24), np.float32)
    p15a = np.zeros((120, 16), np.float32)
    p15b = np.zeros((120, 16), np.float32)
    for r in range(15):
        for c in range(8):
            p5[r * 8 + c, (r // 5) * 8 + c] = 1.0 / 25
            p10a[r * 8 + c, (r // 10) * 8 + c] = 1.0 / 100
            p10b[r * 8 + c, ((r + 15) // 10) * 8 + c] = 1.0 / 100
            p15a[r * 8 + c, c] = 1.0 / 225
            p15b[r * 8 + c, 8 + c] = 1.0 / 225
    return {k: v.astype(np.float16) for k, v in
            dict(p5=p5, p10a=p10a, p10b=p10b, p15a=p15a, p15b=p15b).items()}


# --------------------------------------------------------------------------
# fused device kernel
# --------------------------------------------------------------------------

def build_fused():
    nc = bacc.Bacc("TRN2", target_bir_lowering=False, debug=False,
                   enable_asserts=True, num_devices=NCORES)
    im8 = nc.dram_tensor("im8", [BPC, 3, HP, WP], U8, kind="ExternalInput").ap()
    w16in = nc.dram_tensor("w16pack", [1, PACK_S], F16, kind="ExternalInput").ap()
    w32in = nc.dram_tensor("w32pack", [1, PACK32_S], F32,
                           kind="ExternalInput").ap()

    a1specs = {  # name -> (K, M)
        "l1": (24, 48), "l2": (64, 96),
        "l3a": (128, 96), "l3b": (128, 96),
        "l4a": (128, 96), "l4b": (128, 96),
        "l5": (128, 48),
    }
    a2names = ("c5x", "c5u", "c10x", "c10u", "c15x", "c15u",
               "wg0", "wg1", "wg2", "wm0", "wm1", "wm2")

    attd = nc.dram_tensor("att", [BPC, H, W], U8, kind="ExternalOutput").ap()

    # internal DRAM
    w16stage = nc.dram_tensor("w16stage", [1, PACK_S], F16)
    w16all = nc.dram_tensor("w16all", [NCORES, PACK_S], F16)
    x16 = nc.dram_tensor("x16", [BPC, HP, 8, WP], F16).ap()
    up = {k: nc.dram_tensor(f"up{k}", [BPC, HP, 8, WP], F16).ap()
          for k in (5, 10, 15)}
    upx = {5: nc.dram_tensor("upx5", [BPC, 60, 8, WP], F16).ap(),
           10: nc.dram_tensor("upx10", [BPC, 30, 8, WP], F16).ap(),
           15: nc.dram_tensor("upx15", [BPC, 20, 8, WP], F16).ap()}
    gat = nc.dram_tensor("gat", [BPC, HP, 8, WP], F32).ap()
    ar_in = nc.dram_tensor("ar_in", [8, 2], F32)
    ar_out = nc.dram_tensor("ar_out", [8, 2], F32)

    with tile.TileContext(nc) as tc, ExitStack() as octx:
        nc.sync.dma_start(w16stage.ap()[:], w16in[:])
        nc.gpsimd.collective_compute(
            "AllGather", ALU.bypass,
            replica_groups=[list(range(NCORES))],
            ins=[w16stage.ap()[:].opt()], outs=[w16all.ap()[:].opt()])
        wflat = w16all.ap().rearrange("a s -> (a s)")

        w32flat = w32in.rearrange("a s -> (a s)")
        wp = octx.enter_context(tc.tile_pool(name="wp", bufs=1))

        def wtile(nm, K, C):
            t = wp.tile([K, C], F16, tag=f"w{nm}")
            off = F16_OFFS[nm]
            nc.sync.dma_start(t[:], wflat[off:off + K * C]
                              .rearrange("(k c) -> k c", c=C))
            return t

        def wtile32(nm, K, C):
            t = wp.tile([K, C], F32, tag=f"w{nm}")
            off = F32_OFFS[nm]
            nc.sync.dma_start(t[:], w32flat[off:off + K * C]
                              .rearrange("(k c) -> k c", c=C))
            return t

        W16, B32 = {}, {}
        for nm, (K, M) in a1specs.items():
            W16[nm] = wtile(f"w_{nm}", K, 3 * M)
        for nm in a2names:
            W16[nm] = wtile(f"w_{nm}", 112, 288)
        for nm, K, C in F32_SPECS:
            if nm.startswith("b_"):
                B32[nm[2:]] = wtile32(nm, K, C)
        PM = {nm: wtile(f"pm_{nm}", 120, 24) for nm in ("p5", "p10a", "p10b")}
        PM.update({nm: wtile(f"pm_{nm}", 120, 16) for nm in ("p15a", "p15b")})
        w11 = wtile("w_l11", 112, 36)
        red96 = wtile32("red96", 96, 8)
        bc112 = wtile32("bc112", 8, 336)
        bnwb = wtile32("bnwb", 8, 2)
        b11t = wtile32("b11t", 12, 1)
        zt = wp.tile([16, WP], F16, tag="zt")
        nc.vector.memset(zt[:], 0.0)
        zt32 = wp.tile([8, WP], F32, tag="zt32")
        nc.vector.memset(zt32[:], 0.0)
        acc = wp.tile([96, 2], F32, tag="acc")
        nc.vector.memset(acc[:], 0.0)
        sc = wp.tile([112, 6], F32, tag="sc")

        # ================= phase A1: conv1..conv5 =================
        with ExitStack() as ctx:
            p_im = ctx.enter_context(tc.tile_pool(name="p_im", bufs=4))
            p2 = ctx.enter_context(tc.tile_pool(name="p2", bufs=6))
            p3 = ctx.enter_context(tc.tile_pool(name="p3", bufs=6))
            p4a = ctx.enter_context(tc.tile_pool(name="p4a", bufs=6))
            p4b = ctx.enter_context(tc.tile_pool(name="p4b", bufs=6))
            p5p = ctx.enter_context(tc.tile_pool(name="p5p", bufs=6))
            ps = ctx.enter_context(tc.tile_pool(name="ps", bufs=8, space="PSUM"))
            ev = ctx.enter_context(tc.tile_pool(name="ev", bufs=4))

            for img in range(BPC):
                P2, P3, P4A, P4B, P5 = {}, {}, {}, {}, {}

                def mm3(pt, wtile, K, M, pan, start=True, stop=True):
                    for dx in range(3):
                        nc.tensor.matmul(pt[0:M, :], wtile[:K, dx * M:dx * M + M],
                                         pan[:K, dx:dx + W],
                                         start=(start and dx == 0),
                                         stop=(stop and dx == 2))

                def halo(panels, t, C):
                    pan = panels[t]
                    if t == 0:
                        nc.sync.dma_start(pan[6 * C:7 * C, :], zt[:C, :])
                    else:
                        nc.sync.dma_start(pan[6 * C:7 * C, :],
                                          panels[t - 1][5 * C:6 * C, :])
                    if t == 49:
                        nc.sync.dma_start(pan[7 * C:8 * C, :], zt[:C, :])
                    else:
                        nc.sync.dma_start(pan[7 * C:8 * C, :], panels[t + 1][0:C, :])

                def evac_dve(dst, n, pt, m, btile):
                    nc.vector.tensor_scalar(dst[0:n, 1:301], pt[0:m, :], btile, None,
                                            op0=ALU.add)
                    nc.vector.memset(dst[0:n, 0:1], 0.0)
                    nc.vector.memset(dst[0:n, 301:302], 0.0)

                def evac_act(dst, n, pt, m, btile):
                    nc.scalar.activation(dst[0:n, 1:301], pt[0:m, :], AF.Identity,
                                         bias=btile)
                    nc.vector.memset(dst[0:n, 0:1], 0.0)
                    nc.vector.memset(dst[0:n, 301:302], 0.0)

                def L1(t):
                    pan8 = p_im.tile([24, WP], U8, tag="imp8")
                    nc.sync.dma_start(pan8[:],
                                      im8[img, :, 6 * t:6 * t + 8, :]
                                      .rearrange("c y x -> y c x"))
                    pan = p_im.tile([24, WP], F16, tag="imp")
                    nc.scalar.activation(pan[:], pan8[:], AF.Copy,
                                         bias=-128.0 * S_IM, scale=S_IM)
                    pt = ps.tile([48, W], F32, tag="ps")
                    mm3(pt, W16["l1"], 24, 48, pan)
                    dst = p2.tile([64, WP], F16, tag="p2")
                    P2[t] = dst
                    evac_dve(dst, 48, pt, 48, B32["l1"][:])

                def L2(t):
                    halo(P2, t, 8)
                    pt = ps.tile([96, W], F32, tag="ps")
                    mm3(pt, W16["l2"], 64, 96, P2[t])
                    dst = p3.tile([128, WP], F16, tag="p3")
                    P3[t] = dst
                    evac_act(dst, 96, pt, 96, B32["l2"][:])

                def L3(t):
                    halo(P3, t, 16)
                    pta = ps.tile([96, W], F32, tag="ps")
                    ptb = ps.tile([96, W], F32, tag="ps")
                    mm3(pta, W16["l3a"], 128, 96, P3[t])
                    mm3(ptb, W16["l3b"], 128, 96, P3[t])
                    for nm, pt, pool, store, ed in (("l3a", pta, p4a, P4A, evac_dve),
                                                    ("l3b", ptb, p4b, P4B, evac_act)):
                        dst = pool.tile([128, WP], F16, tag=nm)
                        store[t] = dst
                        ed(dst, 96, pt, 96, B32[nm][:])

                def L4(t):
                    halo(P4A, t, 16)
                    halo(P4B, t, 16)
                    pt = ps.tile([96, W], F32, tag="ps")
                    for bi, (wnm, pan) in enumerate((("l4a", P4A[t]),
                                                     ("l4b", P4B[t]))):
                        for dx in range(3):
                            nc.tensor.matmul(pt[:, :],
                                             W16[wnm][:, dx * 96:dx * 96 + 96],
                                             pan[:128, dx:dx + W],
                                             start=(bi == 0 and dx == 0),
                                             stop=(bi == 1 and dx == 2))
                    dst = p5p.tile([128, WP], F16, tag="p5")
                    P5[t] = dst
                    evac_dve(dst, 96, pt, 96, B32["l4"][:])

                def L5(t):
                    halo(P5, t, 16)
                    pt = ps.tile([48, W], F32, tag="ps")
                    mm3(pt, W16["l5"], 128, 48, P5[t])
                    o = ev.tile([48, WP], F16, tag="xev")
                    nc.vector.tensor_scalar(o[:, 1:301], pt[:, :], B32["l5"][:],
                                            None, op0=ALU.add)
                    nc.vector.memset(o[:, 0:1], 0.0)
                    nc.vector.memset(o[:, 301:302], 0.0)
                    nc.sync.dma_start(x16[img, 6 * t + 1:6 * t + 7, :, :], o[:, :])

                for s in range(0, 58):
                    if s < 50:
                        L1(s)
                    if 0 <= s - 2 < 50:
                        L2(s - 2)
                    if 0 <= s - 4 < 50:
                        L3(s - 4)
                    if 0 <= s - 6 < 50:
                        L4(s - 6)
                    if 0 <= s - 8 < 50:
                        L5(s - 8)
                # zero-pad border rows of x16
                nc.sync.dma_start(x16[img, 0, :, :], zt[:8, :])
                nc.sync.dma_start(x16[img, 301, :, :], zt[:8, :])

        # ================= pools + upsample =================
        with ExitStack() as ctx:
            pin = ctx.enter_context(tc.tile_pool(name="plin", bufs=4))
            psp = ctx.enter_context(tc.tile_pool(name="plps", bufs=2, space="PSUM"))
            pev = ctx.enter_context(tc.tile_pool(name="plev", bufs=4))
            pex = ctx.enter_context(tc.tile_pool(name="plex", bufs=4))

            def expand(pooled, P, X, k, dst_ap):
                """pooled [P, X] f32 -> fp16 [P, 302] X-expanded, DMA to dst."""
                xs = pex.tile([P, WP], F16, tag=f"xs{k}")
                nc.vector.tensor_copy(
                    xs[:, 1:301].rearrange("p (a b) -> p a b", b=k),
                    pooled[:].unsqueeze(2).broadcast_to([P, X, k]))
                nc.vector.memset(xs[:, 0:1], 0.0)
                nc.vector.memset(xs[:, 301:302], 0.0)
                nc.sync.dma_start(dst_ap, xs[:])

            for img in range(BPC):
                prev = None
                for ci in range(20):
                    xt = pin.tile([120, WP], F16, tag="xt")
                    nc.sync.dma_start(xt[:],
                                      x16[img, 1 + 15 * ci:16 + 15 * ci, :, :])
                    # k=5: 3 pooled rows from this tile
                    pt5 = psp.tile([24, WP], F32, tag="ps5")
                    nc.tensor.matmul(pt5[:, :], PM["p5"][:, :], xt[:, :],
                                     start=True, stop=True)
                    pl5 = pev.tile([24, 60], F32, tag="pl5")
                    nc.vector.tensor_reduce(
                        pl5[:],
                        pt5[:, 1:301].rearrange("p (a b) -> p a b", b=5),
                        axis=mybir.AxisListType.X, op=ALU.add)
                    expand(pl5, 24, 60, 5, upx[5][img, 3 * ci:3 * ci + 3, :, :])
                    if ci % 2 == 1:
                        m = ci // 2
                        # k=10: 3 pooled rows from tile pair
                        pt10 = psp.tile([24, WP], F32, tag="ps10")
                        nc.tensor.matmul(pt10[:, :], PM["p10a"][:, :], prev[:, :],
                                         start=True, stop=False)
                        nc.tensor.matmul(pt10[:, :], PM["p10b"][:, :], xt[:, :],
                                         start=False, stop=True)
                        pl10 = pev.tile([24, 30], F32, tag="pl10")
                        nc.vector.tensor_reduce(
                            pl10[:],
                            pt10[:, 1:301].rearrange("p (a b) -> p a b", b=10),
                            axis=mybir.AxisListType.X, op=ALU.add)
                        expand(pl10, 24, 30, 10,
                               upx[10][img, 3 * m:3 * m + 3, :, :])
                        # k=15: 2 pooled rows from tile pair
                        pt15 = psp.tile([16, WP], F32, tag="ps15")
                        nc.tensor.matmul(pt15[:, :], PM["p15a"][:, :], prev[:, :],
                                         start=True, stop=False)
                        nc.tensor.matmul(pt15[:, :], PM["p15b"][:, :], xt[:, :],
                                         start=False, stop=True)
                        pl15 = pev.tile([16, 20], F32, tag="pl15")
                        nc.vector.tensor_reduce(
                            pl15[:],
                            pt15[:, 1:301].rearrange("p (a b) -> p a b", b=15),
                            axis=mybir.AxisListType.X, op=ALU.add)
                        expand(pl15, 16, 20, 15,
                               upx[15][img, 2 * m:2 * m + 2, :, :])
                    prev = xt
                # Y-replication into full up tensors + border rows
                for k in (5, 10, 15):
                    v = up[k][img, 1:301, :, :].rearrange(
                        "(Y j) c x -> Y j c x", j=k)
                    for jy in range(k):
                        nc.sync.dma_start(v[:, jy, :, :], upx[k][img, :, :, :])
                    nc.sync.dma_start(up[k][img, 0, :, :], zt[:8, :])
                    nc.sync.dma_start(up[k][img, 301, :, :], zt[:8, :])

        # ================= phase A2: c5/c10/c15, gated, stats =================
        with ExitStack() as ctx:
            pin = ctx.enter_context(tc.tile_pool(name="pin", bufs=3))
            pc = {k: ctx.enter_context(tc.tile_pool(name=f"pc{k}", bufs=6))
                  for k in (5, 10, 15)}
            ps = ctx.enter_context(tc.tile_pool(name="ps2", bufs=8, space="PSUM"))
            ev = ctx.enter_context(tc.tile_pool(name="ev2", bufs=3))

            for img in range(BPC):
                CP = {5: {}, 10: {}, 15: {}}

                def CL(k, t, panx):
                    panu = pin.tile([112, WP], F16, tag=f"panu{k}")
                    nc.sync.dma_start(panu[:], up[k][img, 12 * t:12 * t + 14, :, :])
                    pt = ps.tile([96, W], F32, tag="ps")
                    for bi, (wnm, pan) in enumerate(((f"c{k}x", panx),
                                                     (f"c{k}u", panu))):
                        for dx in range(3):
                            nc.tensor.matmul(pt[:, :],
                                             W16[wnm][:, dx * 96:dx * 96 + 96],
                                             pan[:, dx:dx + W],
                                             start=(bi == 0 and dx == 0),
                                             stop=(bi == 1 and dx == 2))
                    dst = pc[k].tile([112, WP], F16, tag=f"cp{k}")
                    CP[k][t] = dst
                    nc.vector.tensor_scalar(dst[0:96, 1:301], pt[:, :],
                                            B32[f"c{k}"][:], None, op0=ALU.add)
                    nc.vector.memset(dst[0:96, 0:1], 0.0)
                    nc.vector.memset(dst[0:96, 301:302], 0.0)

                def halo12(panels, t):
                    pan = panels[t]
                    if t == 0:
                        nc.sync.dma_start(pan[96:104, :], zt[:8, :])
                    else:
                        nc.sync.dma_start(pan[96:104, :], panels[t - 1][88:96, :])
                    if t == 24:
                        nc.sync.dma_start(pan[104:112, :], zt[:8, :])
                    else:
                        nc.sync.dma_start(pan[104:112, :], panels[t + 1][0:8, :])

                def GATED(t):
                    for k in (5, 10, 15):
                        halo12(CP[k], t)
                    ptg = ps.tile([96, W], F32, tag="ps")
                    ptm = ps.tile([96, W], F32, tag="ps")
                    for pt, pfx in ((ptg, "wg"), (ptm, "wm")):
                        for bi, k in enumerate((5, 10, 15)):
                            wtile = W16[f"{pfx}{bi}"]
                            for dx in range(3):
                                nc.tensor.matmul(pt[:, :],
                                                 wtile[:, dx * 96:dx * 96 + 96],
                                                 CP[k][t][:, dx:dx + W],
                                                 start=(bi == 0 and dx == 0),
                                                 stop=(bi == 2 and dx == 2))
                    s = ev.tile([96, W], F32, tag="sig")
                    nc.scalar.activation(s[:, :], ptm[:, :], AF.Sigmoid,
                                         bias=B32["wm"][:])
                    g = ev.tile([96, W], F32, tag="gg")
                    nc.vector.tensor_scalar(g[:, :], ptg[:, :], B32["wg"][:], None,
                                            op0=ALU.add)
                    gv = ev.tile([96, WP], F32, tag="gv")
                    nc.vector.tensor_tensor(gv[:, 1:301], g[:, :], s[:, :],
                                            op=ALU.mult)
                    nc.vector.memset(gv[:, 0:1], 0.0)
                    nc.vector.memset(gv[:, 301:302], 0.0)
                    nc.sync.dma_start(gat[img, 12 * t + 1:12 * t + 13, :, :],
                                      gv[:, :])
                    red = ev.tile([96, 2], F32, tag="red")
                    nc.vector.tensor_reduce(red[:, 0:1], gv[:, 1:301],
                                            axis=mybir.AxisListType.X, op=ALU.add)
                    sq = ev.tile([96, W], F32, tag="sq")
                    nc.vector.tensor_tensor(sq[:, :], gv[:, 1:301], gv[:, 1:301],
                                            op=ALU.mult)
                    nc.vector.tensor_reduce(red[:, 1:2], sq[:, :],
                                            axis=mybir.AxisListType.X, op=ALU.add)
                    nc.vector.tensor_tensor(acc[:, :], acc[:, :], red[:, :],
                                            op=ALU.add)

                for u in range(0, 27):
                    if u < 25:
                        panx = pin.tile([112, WP], F16, tag="panx")
                        nc.sync.dma_start(panx[:],
                                          x16[img, 12 * u:12 * u + 14, :, :])
                        for k in (5, 10, 15):
                            CL(k, u, panx)
                    if 0 <= u - 2 < 25:
                        GATED(u - 2)
                nc.sync.dma_start(gat[img, 0, :, :], zt32[:, :])
                nc.sync.dma_start(gat[img, 301, :, :], zt32[:, :])

        # ================= stats allreduce + BN fold =================
        with ExitStack() as ctx:
            stp = ctx.enter_context(tc.tile_pool(name="stp", bufs=1))
            pss = ctx.enter_context(tc.tile_pool(name="pss", bufs=2, space="PSUM"))
            p8 = pss.tile([8, 2], F32, tag="p8")
            nc.tensor.matmul(p8[:, :], red96[:, :], acc[:, :],
                             start=True, stop=True)
            st8 = stp.tile([8, 2], F32, tag="st8")
            nc.vector.tensor_copy(st8[:], p8[:])
            nc.sync.dma_start(ar_in.ap()[:], st8[:])
            nc.gpsimd.collective_compute(
                "AllReduce", ALU.add,
                replica_groups=[list(range(NCORES))],
                ins=[ar_in.ap()[:].opt()], outs=[ar_out.ap()[:].opt()])
            g8 = stp.tile([8, 2], F32, tag="g8")
            nc.sync.dma_start(g8[:], ar_out.ap()[:])
            m8 = stp.tile([8, 2], F32, tag="m8")
            nc.vector.tensor_scalar(m8[:], g8[:], 1.0 / NPIX, None, op0=ALU.mult)
            msq = stp.tile([8, 1], F32, tag="msq")
            nc.vector.tensor_tensor(msq[:], m8[:, 0:1], m8[:, 0:1], op=ALU.mult)
            var = stp.tile([8, 1], F32, tag="var")
            nc.vector.scalar_tensor_tensor(var[:], m8[:, 1:2], float(EPS), msq[:],
                                           op0=ALU.add, op1=ALU.subtract)
            zb = stp.tile([8, 1], F32, tag="zb")
            nc.vector.memset(zb[:], 0.0)
            std = stp.tile([8, 1], F32, tag="std")
            nc.scalar.activation(std[:], var[:], AF.Sqrt, bias=zb[:])
            inv = stp.tile([8, 1], F32, tag="inv")
            nc.vector.reciprocal(inv[:], std[:])
            so8 = stp.tile([8, 2], F32, tag="so8")
            nc.vector.tensor_tensor(so8[:, 0:1], inv[:], bnwb[:, 0:1], op=ALU.mult)
            tmp = stp.tile([8, 1], F32, tag="tmp")
            nc.vector.tensor_tensor(tmp[:], m8[:, 0:1], so8[:, 0:1], op=ALU.mult)
            nc.vector.tensor_tensor(so8[:, 1:2], bnwb[:, 1:2], tmp[:],
                                    op=ALU.subtract)
            for j in range(3):
                pb = pss.tile([112, 2], F32, tag="pb")
                nc.tensor.matmul(pb[:, :], bc112[:, 112 * j:112 * j + 112],
                                 so8[:, :], start=True, stop=True)
                nc.vector.tensor_copy(sc[:, 2 * j:2 * j + 2], pb[:])

        # ================= phase B: 8->1 conv, sigmoid, blend =================
        with ExitStack() as ctx:
            pin = ctx.enter_context(tc.tile_pool(name="pinb", bufs=4))
            ps = ctx.enter_context(tc.tile_pool(name="psb", bufs=4, space="PSUM"))
            ev = ctx.enter_context(tc.tile_pool(name="evb", bufs=4))

            for img in range(BPC):
                for t in range(25):
                    gp32 = pin.tile([112, WP], F32, tag="gp32")
                    nc.sync.dma_start(gp32[:], gat[img, 12 * t:12 * t + 14, :, :])
                    gn = pin.tile([112, WP], F16, tag="gn")
                    j = 1 if t == 0 else (2 if t == 24 else 0)
                    nc.vector.tensor_scalar(gn[:, 1:301], gp32[:, 1:301],
                                            sc[:, 2 * j:2 * j + 1],
                                            sc[:, 2 * j + 1:2 * j + 2],
                                            op0=ALU.mult, op1=ALU.add)
                    nc.vector.memset(gn[:, 0:1], 0.0)
                    nc.vector.memset(gn[:, 301:302], 0.0)
                    ptz = ps.tile([12, W], F32, tag="ps")
                    for dx in range(3):
                        nc.tensor.matmul(ptz[:, :], w11[:, dx * 12:dx * 12 + 12],
                                         gn[:, dx:dx + W],
                                         start=(dx == 0), stop=(dx == 2))
                    sg = ev.tile([12, W], F16, tag="sg")
                    nc.scalar.activation(sg[:, :], ptz[:, :], AF.Sigmoid,
                                         bias=b11t[:])
                    q = ev.tile([12, W], U8, tag="q")
                    nc.vector.tensor_scalar(q[:, :], sg[:, :], 255.0, 0.5,
                                            op0=ALU.mult, op1=ALU.add)
                    nc.sync.dma_start(attd[img, 12 * t:12 * t + 12, :], q[:, :])
    nc.finalize()
    return nc


# --------------------------------------------------------------------------
# cached jit dispatcher (axon / PJRT)
# --------------------------------------------------------------------------

def make_dispatcher(nc):
    import jax
    import jax.numpy as jnp
    from jax.sharding import Mesh, PartitionSpec, NamedSharding
    from jax.experimental.shard_map import shard_map

    bass2jax.install_neuronx_cc_hook()
    partition_name = nc.partition_id_tensor.name if nc.partition_id_tensor else None
    in_names, out_names, out_avals, zero_shapes = [], [], [], []
    for alloc in nc.m.functions[0].allocations:
        if not isinstance(alloc, mybir.MemoryLocationSet):
            continue
        name = alloc.memorylocations[0].name
        if alloc.kind == "ExternalInput":
            if name != partition_name:
                in_names.append(name)
        elif alloc.kind == "ExternalOutput":
            out_names.append(name)
            shape = tuple(alloc.tensor_shape)
            dtype = mybir.dt.np(alloc.dtype)
            out_avals.append(jax.core.ShapedArray(shape, dtype))
            zero_shapes.append((shape, dtype))
    n_params = len(in_names)
    n_outs = len(out_avals)
    all_in_names = in_names + out_names + ([partition_name] if partition_name else [])
    donate = tuple(range(n_params, n_params + n_outs))

    def _body(*args):
        operands = list(args)
        if partition_name is not None:
            operands.append(bass2jax.partition_id_tensor())
        outs = bass2jax._bass_exec_p.bind(
            *operands, out_avals=tuple(out_avals), in_names=tuple(all_in_names),
            out_names=tuple(out_names), lowering_input_output_aliases=(),
            sim_require_finite=True, sim_require_nnan=True, nc=nc)
        return tuple(outs)

    devices = jax.devices()[:NCORES]
    mesh = Mesh(np.asarray(devices), ("core",))
    in_specs = (PartitionSpec("core"),) * (n_params + n_outs)
    out_specs = (PartitionSpec("core"),) * len(out_names)
    sharded = jax.jit(shard_map(_body, mesh=mesh, in_specs=in_specs,
                                out_specs=out_specs, check_rep=False),
                      donate_argnums=donate, keep_unused=True)
    zshard = NamedSharding(mesh, PartitionSpec("core"))
    zmaker = jax.jit(
        lambda: tuple(jnp.zeros((NCORES * s[0], *s[1:]), d)
                      for s, d in zero_shapes),
        out_shardings=tuple([zshard] * n_outs))
    return sharded, zmaker, in_names, out_names


# --------------------------------------------------------------------------
# host orchestration
# --------------------------------------------------------------------------

def _weight_inputs(w1, b1, w2, b2, w3, b3, w4, b4, w5, b5,
                   wc5, bc5, wc10, bc10, wc15, bc15, wg, bg, wm, bm,
                   bn_w, bn_b, w11, b11, gamma):
    d = {
        "w_l1": band_lhs(w1, 6, range(3), range(8)).astype(np.float16),
        "w_l2": band_lhs(w2, 6, range(8), range(16), perm=True).astype(np.float16),
        "w_l3a": band_lhs(w3, 6, range(16), range(16), perm=True).astype(np.float16),
        "w_l3b": band_lhs(w3, 6, range(16), range(16, 32), perm=True).astype(np.float16),
        "w_l4a": band_lhs(w4, 6, range(16), range(16), perm=True).astype(np.float16),
        "w_l4b": band_lhs(w4, 6, range(16, 32), range(16), perm=True).astype(np.float16),
        "w_l5": band_lhs(w5, 6, range(16), range(8), perm=True).astype(np.float16),
        "b_l1": tile_bias(b1, 6), "b_l2": tile_bias(b2, 6),
        "b_l3a": tile_bias(np.asarray(b3)[:16], 6),
        "b_l3b": tile_bias(np.asarray(b3)[16:], 6),
        "b_l4": tile_bias(b4, 6), "b_l5": tile_bias(b5, 6),
    }
    for k, wc, bcv in ((5, wc5, bc5), (10, wc10, bc10), (15, wc15, bc15)):
        d[f"w_c{k}x"] = band_lhs(wc, 12, range(0, 8), range(8)).astype(np.float16)
        d[f"w_c{k}u"] = band_lhs(wc, 12, range(8, 16), range(8)).astype(np.float16)
        d[f"b_c{k}"] = tile_bias(bcv, 12)
    for pfx, wv, bv in (("wg", wg, bg), ("wm", wm, bm)):
        for bi in range(3):
            d[f"w_{pfx}{bi}"] = band_lhs(
                wv, 12, range(8 * bi, 8 * bi + 8), range(8),
                perm=True).astype(np.float16)
        d[f"b_{pfx}"] = tile_bias(bv, 12)
    for nm, v in pool_mats().items():
        d[f"pm_{nm}"] = v
    d["w_l11"] = band_lhs(np.asarray(w11, np.float32), 12, range(8),
                          range(1)).astype(np.float16)
    red96 = np.zeros((96, 8), np.float32)
    for y in range(12):
        for c in range(8):
            red96[y * 8 + c, c] = 1.0
    d["red96"] = red96
    bc112 = np.zeros((8, 3, 112), np.float32)
    for y in range(14):
        for c in range(8):
            bc112[c, :, y * 8 + c] = 1.0
    bc112[:, 1, 0:8] = 0.0      # t=0: top halo row is zero padding
    bc112[:, 2, 104:112] = 0.0  # t=24: bottom halo row is zero padding
    d["bc112"] = bc112.reshape(8, 3 * 112)
    d["bnwb"] = np.stack([np.asarray(bn_w, np.float32),
                          np.asarray(bn_b, np.float32)], axis=1)
    d["b11t"] = np.full((12, 1), float(np.asarray(b11).reshape(-1)[0]),
                        np.float32)
    return d


def kernel(im, w1, b1, w2, b2, w3, b3, w4, b4, w5, b5,
           wc5, bc5, wc10, bc10, wc15, bc15,
           wg, bg, wm, bm, bn_w, bn_b, w11, b11, gamma):
    im = np.asarray(im, np.float32)
    args = [np.asarray(a, np.float32) for a in
            (w1, b1, w2, b2, w3, b3, w4, b4, w5, b5,
             wc5, bc5, wc10, bc10, wc15, bc15, wg, bg, wm, bm)]

    if "nc" not in _CACHE:
        _CACHE["nc"] = build_fused()
        (_CACHE["sharded"], _CACHE["zmaker"],
         _CACHE["in_names"], _CACHE["out_names"]) = make_dispatcher(_CACHE["nc"])
    sharded, zmaker = _CACHE["sharded"], _CACHE["zmaker"]

    import jax
    from jax.sharding import Mesh, PartitionSpec, NamedSharding
    if "imsh" not in _CACHE:
        from concurrent.futures import ThreadPoolExecutor
        _CACHE["devices"] = jax.devices()[:NCORES]
        mesh = Mesh(np.asarray(_CACHE["devices"]), ("core",))
        _CACHE["imsh"] = NamedSharding(mesh, PartitionSpec("core"))
        _CACHE["pool"] = ThreadPoolExecutor(NCORES)

    # quantize + upload the image per shard; threaded so all 8 uploads are
    # in flight within ~10ms (np.clip releases the GIL)
    def quant_up(c):
        im8c = np.full((BPC, 3, HP, WP), 128, np.uint8)
        np.clip(im[BPC * c:BPC * (c + 1)] * (1.0 / S_IM) + 128.5, 0, 255,
                out=im8c[:, :, 1:301, 1:301], casting="unsafe")
        return jax.device_put(im8c, _CACHE["devices"][c])

    shards = list(_CACHE["pool"].map(quant_up, range(NCORES)))
    im8_dev = jax.make_array_from_single_device_arrays(
        (NCORES * BPC, 3, HP, WP), _CACHE["imsh"], shards)

    wd = _weight_inputs(*args, bn_w, bn_b, w11, b11, gamma)
    pack = np.zeros(NCORES * PACK_S, np.float16)
    for nm, K, C in F16_SPECS:
        o = F16_OFFS[nm]
        pack[o:o + K * C] = wd.pop(nm).ravel()
    p32 = np.concatenate([np.asarray(wd[nm], np.float32).ravel()
                          for nm, _, _ in F32_SPECS])
    full = {"im8": im8_dev, "w16pack": pack.reshape(NCORES, PACK_S),
            "w32pack": np.tile(p32, (NCORES, 1))}

    ins = [full[nm] for nm in _CACHE["in_names"]]
    zeros = zmaker()
    outs = sharded(*ins, *zeros)
    att = outs[0]

    gamma_v = float(np.asarray(gamma).reshape(-1)[0])
    sc_v, off_v = gamma_v / 255.0, 1.0 - gamma_v
    out = np.empty((32, 3, H, W), np.float32)

    def work(shard):
        sl = shard.index[0]
        qx = np.asarray(shard.data).reshape(-1, H, W)
        f = qx.astype(np.float32)
        f *= sc_v
        f += off_v
        np.multiply(im[sl], f[:, None, :, :], out=out[sl])

    list(_CACHE["pool"].map(work, att.addressable_shards))
    return out
